# revision 32
# baseline (speedup 1.0000x reference)
"""Multi-head attention (B=4, N=1568, C=768, H=12) on 8 TRN2 NeuronCores.

Sharding: query-parallel. Core c handles batch b = c // 2 and query half
half = c % 2 (784 query tokens). Each core computes K/V projections for the
full 1568 tokens of its batch (duplicated across the pair), Q projection
for its 784 tokens, full attention for all 12 heads over its queries, and
the output projection. No cross-core communication.

Host-side tricks:
  - tokens are rotated per core so its own query half sits at columns 0:784
    of xT; the key order is then a (core-dependent) permutation, which
    softmax attention is invariant to.
  - v_bias is folded into the projection bias.
  - the softmax 1/sqrt(D) scale is folded into the exp activation's scale.

Device schedule (per core), heads in PAIRS (2ft, 2ft+1):
  - inputs arrive via a few LARGE consolidated DMAs (dma submission on the
    sync engine costs ~0.6us each); SBUF input tiles are split exactly at
    DMA boundaries so compute depends only on the pieces it reads
  - the head phase computes K(0), K(1), Q(0) back-to-back so the PE has no
    idle window (no HAM re-throttle) while the remaining weights stream in
  - flat software pipeline over (pair, key-tile) steps: scores(s) are
    emitted 2 steps ahead of PV(s), so the ACT-engine exp overlaps PV/V/
    projection matmuls and the PE never queue-blocks on the scalar engine
  - PV ("form B"): ex q-tiles are the matmul STATIONARY (128-wide tiles so
    the compiler's fast-weight-load halves the LDWEIGHTS cost; head A and
    head B tiles interleave so the weight-load pipeline never drains) and
    [V | ones] (65 cols) streams; col 64 gives the softmax denominator
    per-partition, normalized by a cheap reciprocal + tensor_scalar mult
  - normalize is split: the DVE multiply chain runs one step before the PE
    transposes so the PE FIFO never waits on the vector engine
  - output projection is INCREMENTAL: pair j's 12 rank-128 contributions
    are spread over later steps (a deque carries the overflow into the
    next pair, never scheduled before that pair's attn tile is written to
    keep the PE FIFO deadlock-free) and accumulated into SBUF f32
    accumulators on the vector engine (bias folded in at j==0), so only
    14 matmuls + two output DMAs remain after the last pair
  - per-step PSUM-scratch (psA, 2 banks) users are scheduled at most ~2
    allocations per step: V at tt 0/4/8/12, single K chunks at 2/6/9/11,
    Q chunks at 1/5, transposes at 3, partials fill the gaps
"""

import numpy as np
import ml_dtypes

B, N, C = 4, 1568, 768
H = 12
D = 64
NQ = N // 2          # 784 queries per core
SCALE = D ** -0.5
N_CORES = 8
KT = [128] * 12 + [32]          # key tiles (sum = 1568)
TCH = [(0, 392), (392, 392), (784, 392), (1176, 392)]  # token chunks (K/Q proj)
# query tiles for form-B PV: (ex column offset, width, output q offset).
# ex is laid out head-contiguous (A: cols 0:784, B: 784:1568); 6x128 + 16
# (128-wide stationaries trigger the compiler's fast weight load)
QT_A = [(128 * i, 128, 128 * i) for i in range(6)] + [(768, 16, 768)]
QT_B = [(784 + c, w, o) for (c, w, o) in QT_A]
# proj query chunks
PCH = [(0, 512), (512, 272)]
# how many pending proj partials to pop at each step tt. Slot 1 may only
# serve partials carried over from the PREVIOUS pair (attn of the current
# pair's j=p-1 is written at tt==3); capacity/pair = 12 with 1 carried.
PARTIAL_POPS = {1: 1, 4: 1, 5: 1, 6: 2, 7: 2, 8: 1, 10: 2, 12: 2}
PARTIAL_SAFE_EARLY = (1,)     # slots that may only serve carried partials

_cache = {}


def _build_program():
    import concourse.mybir as mybir
    from concourse import bacc
    from concourse.tile import TileContext

    f32 = mybir.dt.float32
    bf16 = mybir.dt.bfloat16
    Exp = mybir.ActivationFunctionType.Exp

    nc = bacc.Bacc("TRN2", target_bir_lowering=False, debug=False,
                   num_devices=N_CORES)

    xT_d = nc.dram_tensor("xT", [C, N], bf16, kind="ExternalInput")
    wqk_d = nc.dram_tensor("wqk", [C, 2 * C], bf16, kind="ExternalInput")
    wv_d = nc.dram_tensor("wv", [C, C], bf16, kind="ExternalInput")
    wp_d = nc.dram_tensor("wproj", [C, C], bf16, kind="ExternalInput")
    qb_d = nc.dram_tensor("qb", [128, 6], f32, kind="ExternalInput")
    pb_d = nc.dram_tensor("pb", [128, 6], f32, kind="ExternalInput")
    id_d = nc.dram_tensor("ident", [128, 128], bf16, kind="ExternalInput")
    out_d = nc.dram_tensor("outT", [C, NQ], f32, kind="ExternalOutput")

    with TileContext(nc) as tc:
        persist_cm = tc.tile_pool(name="persist", bufs=1)
        persist = persist_cm.__enter__()
        kT = [persist.tile([128, N], bf16, tag=f"kT{j}", name=f"kT{j}")
              for j in range(6)]
        qT = [persist.tile([128, NQ], bf16, tag=f"qT{j}", name=f"qT{j}")
              for j in range(6)]
        # V for all 13 key tiles: [tt][pair][head-of-pair][65] along free dim
        vbig = persist.tile([128, 13 * 780], bf16, tag="vbig", name="vbig")
        attn = [persist.tile([128, NQ], bf16, tag=f"at{j}", name=f"at{j}")
                for j in range(6)]
        # incremental output-projection accumulators (f32); two tiles of
        # three out-blocks each so the two output DMAs depend on halves
        acc_t = [persist.tile([128, 3 * NQ], f32, tag=f"ac{h}", name=f"ac{h}")
                 for h in range(2)]
        acc_v = [t.rearrange("p (o n) -> p o n", o=3) for t in acc_t]
        acc = [acc_v[ot // 3][:, ot % 3, :] for ot in range(6)]
        qb_sb = persist.tile([128, 6], f32, tag="qb")
        pb_sb = persist.tile([128, 6], f32, tag="pb")
        id_sb = persist.tile([128, 128], bf16, tag="ident")

        # input tiles split exactly at DMA granularity
        phA_cm = tc.tile_pool(name="phA", bufs=1)
        phA = phA_cm.__enter__()
        xT_t = [phA.tile([128, N], bf16, tag="xTa", name="xTsa"),
                phA.tile([128, N], bf16, tag="xTb", name="xTsb"),
                phA.tile([128, 2 * N], bf16, tag="xTc", name="xTsc"),
                phA.tile([128, 2 * N], bf16, tag="xTd", name="xTsd")]
        xT_v = [t.rearrange("p (j n) -> p j n", j=max(1, t.shape[1] // N))
                for t in xT_t]
        _xmap = [(0, 0), (1, 0), (2, 0), (2, 1), (3, 0), (3, 1)]
        xT = [xT_v[a][:, b, :] for (a, b) in _xmap]
        wqkK_t = [phA.tile([128, 3 * C], bf16, tag=f"wK{h}", name=f"wKs{h}")
                  for h in range(2)]
        wqkK_v = [t.rearrange("p (j n) -> p j n", j=3) for t in wqkK_t]
        wK = [wqkK_v[j // 3][:, j % 3, :] for j in range(6)]
        wqkQ_t = [phA.tile([128, 3 * C], bf16, tag=f"wQ{h}", name=f"wQs{h}")
                  for h in range(2)]
        wqkQ_v = [t.rearrange("p (j n) -> p j n", j=3) for t in wqkQ_t]
        wQ = [wqkQ_v[j // 3][:, j % 3, :] for j in range(6)]
        wv_big = phA.tile([128, 6 * C], bf16, tag="wv", name="wvs")
        wvv = wv_big.rearrange("p (j n) -> p j n", j=6)
        wv = [wvv[:, j, :] for j in range(6)]
        wp_big = phA.tile([128, 6 * C], bf16, tag="wp", name="wps")
        wpv = wp_big.rearrange("p (j n) -> p j n", j=6)
        wp_sb = [wpv[:, j, :] for j in range(6)]

        def dma_rows(dst_view, dram, r0, r1, c0=None, c1=None):
            src = dram[r0:r1, :] if c0 is None else dram[r0:r1, c0:c1]
            nc.sync.dma_start(
                out=dst_view, in_=src.rearrange("(j p) n -> p j n", p=128))

        # DMA order = consumption order; few large transfers (bandwidth
        # bound) instead of many small ones (submission bound). The first
        # xT block is small so the HAM warmup starts as early as possible.
        dma_rows(xT_v[0], xT_d, 0, 128)
        dma_rows(wqkK_v[0], wqk_d, 0, 384, C, 2 * C)
        dma_rows(xT_v[1], xT_d, 128, 256)
        dma_rows(xT_v[2], xT_d, 256, 512)
        dma_rows(wqkK_v[1], wqk_d, 384, 768, C, 2 * C)
        dma_rows(xT_v[3], xT_d, 512, 768)
        dma_rows(wqkQ_v[0], wqk_d, 0, 384, 0, C)
        dma_rows(wqkQ_v[1], wqk_d, 384, 768, 0, C)
        nc.sync.dma_start(out=qb_sb, in_=qb_d[:])
        nc.sync.dma_start(out=id_sb, in_=id_d[:])
        nc.sync.dma_start(out=pb_sb, in_=pb_d[:])
        dma_rows(wvv[:, :, :], wv_d, 0, 768)   # lands before the first V use
        dma_rows(wpv[:, :, :], wp_d, 0, 768)   # needed ~1 pair in
        # ones columns: every 65th col of vbig starting at 64
        nc.vector.memset(
            vbig.rearrange("p (t e) -> p t e", e=65)[:, :, 64:65], 1.0)

        # PSUM pools: sJ0 2 + sJ1 2 + po 2 + psA 2 = 8 banks
        psS_cm = tc.tile_pool(name="psS", bufs=1, space="PSUM")
        psS = psS_cm.__enter__()
        psO_cm = tc.tile_pool(name="psO", bufs=1, space="PSUM")
        psO = psO_cm.__enter__()
        psA_cm = tc.tile_pool(name="psA", bufs=2, space="PSUM")
        psA = psA_cm.__enter__()
        phB_cm = tc.tile_pool(name="phB", bufs=5)
        phB = phB_cm.__enter__()
        phBn_cm = tc.tile_pool(name="phBn", bufs=3)
        phBn = phBn_cm.__enter__()

        def emit_k(ft, chunks):
            # chunk-group inner loop: consecutive matmuls share the wqk
            # stationary so later chunks skip their weight load
            pss = [psA.tile([128, 512], f32, tag="psA", name=f"k{ft}_{ci}")
                   for ci in chunks]
            for j in range(6):
                for ps, ci in zip(pss, chunks):
                    (t0, tw) = TCH[ci]
                    nc.tensor.matmul(
                        ps[:, 0:tw],
                        wK[j][:, ft * 128:ft * 128 + 128],
                        xT[j][:, t0:t0 + tw],
                        start=(j == 0), stop=(j == 5),
                        skip_group_check=True,
                    )
            for ps, ci in zip(pss, chunks):
                (t0, tw) = TCH[ci]
                nc.vector.tensor_copy(kT[ft][:, t0:t0 + tw], ps[:, 0:tw])

        def emit_q(ft, chunks):
            pss = [psA.tile([128, 512], f32, tag="psA", name=f"q{ft}_{ci}")
                   for ci in chunks]
            for j in range(6):
                for ps, ci in zip(pss, chunks):
                    (t0, tw) = TCH[ci]
                    nc.tensor.matmul(
                        ps[:, 0:tw],
                        wQ[j][:, ft * 128:ft * 128 + 128],
                        xT[j][:, t0:t0 + tw],
                        start=(j == 0), stop=(j == 5),
                        skip_group_check=True,
                    )
            for ps, ci in zip(pss, chunks):
                (t0, tw) = TCH[ci]
                nc.vector.tensor_scalar(
                    out=qT[ft][:, t0:t0 + tw], in0=ps[:, 0:tw],
                    scalar1=qb_sb[:, ft:ft + 1], scalar2=None,
                    op0=mybir.AluOpType.add,
                )

        def emit_v(p, tt0):
            # V for pair p, key tiles tt0..tt0+3 (4-tile batch), into one
            # psA tile then one strided eviction into vbig
            tts = [t for t in range(tt0, min(tt0 + 4, 13))]
            ps = psA.tile([128, 512], f32, tag="psA", name=f"v{p}_{tt0}")
            for i, tt in enumerate(tts):
                mt = KT[tt]
                for j in range(6):
                    nc.tensor.matmul(
                        ps[0:mt, i * 128:i * 128 + 128],
                        xT[j][:, tt * 128:tt * 128 + mt],
                        wv[j][:, p * 128:(p + 1) * 128],
                        start=(j == 0 and i == 0),
                        stop=(j == 5 and i == len(tts) - 1),
                        skip_group_check=True,
                    )
            src = ps.rearrange("q (i h e) -> q i h e", i=4, h=2)[
                0:128, 0:len(tts), :, :]
            dst = vbig.rearrange("q (t pp h e) -> q t pp h e", pp=6, h=2, e=65)[
                0:128, tt0:tt0 + len(tts), p, :, 0:64]
            nc.vector.tensor_copy(dst, src)

        def emit_scores(ft, tt):
            mt = KT[tt]
            sJ0 = psS.tile([128, 1024], f32, tag="sJ0", name=f"s0_{ft}_{tt}")
            sJ1 = psS.tile([128, 1024], f32, tag="sJ1", name=f"s1_{ft}_{tt}")
            ks = slice(tt * 128, tt * 128 + mt)
            nc.tensor.matmul(sJ0[0:mt, 0:512], kT[ft][0:64, ks],
                             qT[ft][0:64, 0:512], start=True, stop=True)
            nc.tensor.matmul(sJ1[0:mt, 0:272], kT[ft][0:64, ks],
                             qT[ft][0:64, 512:784], start=True, stop=True)
            nc.tensor.matmul(sJ0[0:mt, 512:1024], kT[ft][64:128, ks],
                             qT[ft][64:128, 0:512], start=True, stop=True)
            nc.tensor.matmul(sJ1[0:mt, 512:784], kT[ft][64:128, ks],
                             qT[ft][64:128, 512:784], start=True, stop=True)
            # ex layout: head A = cols 0:784, head B = cols 784:1568
            ex = phB.tile([128, 1568], bf16, tag="ex", name=f"ex{ft}_{tt}")
            ex2 = ex.rearrange("p (b q) -> p b q", b=2)
            nc.scalar.activation(out=ex2[0:mt, :, 0:512],
                                 in_=sJ0.rearrange("p (b q) -> p b q", b=2)[0:mt],
                                 func=Exp, scale=SCALE)
            nc.scalar.activation(
                out=ex2[0:mt, :, 512:784],
                in_=sJ1.rearrange("p (b q) -> p b q", b=2)[0:mt, :, 0:272],
                func=Exp, scale=SCALE)
            return ex

        def emit_pv(p, tt, ex, po_pair):
            mt = KT[tt]
            v5 = vbig.rearrange("q (t pp h e) -> q t pp h e", pp=6, h=2, e=65)
            vh = [v5[0:mt, tt, p, hh, :] for hh in range(2)]
            # interleave head A / head B q-tiles so the PE's background
            # weight-load pipeline never drains at a head boundary
            for i in range(7):
                for hh in range(2):
                    (c0, qw, _) = (QT_A if hh == 0 else QT_B)[i]
                    nc.tensor.matmul(
                        po_pair[hh][0:qw, i * 65:(i + 1) * 65],
                        ex[0:mt, c0:c0 + qw],
                        vh[hh],
                        start=(tt == 0 and i == 0),
                        stop=(tt == 12 and i == 6),
                        skip_group_check=True,
                    )

        def evict_po(p, po_pair):
            # the last pair's eviction goes on the (by then idle) scalar
            # engine, freeing the vector engine for the normalize chain; in
            # steady state the scalar engine is busy with exps (strict FIFO
            # queue -- a copy there would delay them), so DVE does it
            pz_pair = []
            for hh in range(2):
                pz = phBn.tile([128, 455], f32, tag=f"pz{hh}",
                               name=f"pz{p}_{hh}")
                if p == 5:
                    nc.scalar.copy(pz, po_pair[hh][:, 0:455])
                else:
                    nc.vector.tensor_copy(pz, po_pair[hh][:, 0:455])
                pz_pair.append(pz)
            return pz_pair

        def emit_norm_mul(p, pz_pair):
            # pz[q, 65i:65i+64] = PV, pz[q, 65i+64] = denominator (per q!)
            aq_pair = []
            for hh in range(2):
                pz = pz_pair[hh]
                qts = QT_A if hh == 0 else QT_B
                rec = phBn.tile([128, 7], f32, tag=f"rec{hh}", name=f"rc{p}_{hh}")
                nc.vector.reciprocal_approx_fast(
                    out=rec,
                    in_=pz.rearrange("q (i e) -> q i e", e=65)[:, 0:7, 64])
                aq = phBn.tile([128, 448], bf16, tag=f"aq{hh}",
                               name=f"aq{p}_{hh}")
                # one multiply for all 7 q-tiles: the reciprocal broadcasts
                # across each tile's 64 feature columns via a 0-stride AP
                # (garbage beyond the 16 valid partitions of the last tile
                # is never read downstream)
                nc.vector.tensor_tensor(
                    aq.rearrange("q (i e) -> q i e", e=64),
                    pz.rearrange("q (i e) -> q i e", e=65)[:, :, 0:64],
                    rec.rearrange("q (i o) -> q i o", o=1)
                    .broadcast_to([128, 7, 64]),
                    mybir.AluOpType.mult)
                aq_pair.append(aq)
            return aq_pair

        def emit_norm_transpose(p, aq_pair):
            # transpose [q, 64] tiles back to feature-major via PE
            for hh in range(2):
                ft, fo = p, hh * 64
                aq = aq_pair[hh]
                qts = QT_A if hh == 0 else QT_B
                for g0, gn, q0 in ((0, 4, 0), (4, 3, 512)):
                    pt = psA.tile([128, 512], bf16, tag="psA",
                                  name=f"tr{p}_{hh}_{g0}")
                    for i in range(g0, g0 + gn):
                        qw = qts[i][1]
                        qo = qts[i][2] - q0
                        nc.tensor.transpose(
                            pt[0:64, qo:qo + qw],
                            aq[0:qw, i * 64:i * 64 + 64],
                            id_sb[0:qw, 0:qw])
                    gw = qts[g0 + gn - 1][2] + qts[g0 + gn - 1][1] - q0
                    nc.vector.tensor_copy(
                        attn[ft][fo:fo + 64, q0:q0 + gw], pt[0:64, 0:gw])

        def emit_partial(j, k):
            # proj contribution of attn feature-tile j to out tile ot,
            # query chunk (q0, qw); accumulated on DVE into acc[ot]
            ot, ci = k // 2, k % 2
            (q0, qw) = PCH[ci]
            ps = psA.tile([128, 512], f32, tag="psA", name=f"pp{j}_{k}")
            nc.tensor.matmul(
                ps[:, 0:qw],
                wp_sb[j][:, ot * 128:(ot + 1) * 128],
                attn[j][:, q0:q0 + qw],
                start=True, stop=True,
                skip_group_check=True,
            )
            if j == 0:
                nc.vector.tensor_scalar(
                    out=acc[ot][:, q0:q0 + qw], in0=ps[:, 0:qw],
                    scalar1=pb_sb[:, ot:ot + 1], scalar2=None,
                    op0=mybir.AluOpType.add,
                )
            else:
                nc.vector.tensor_add(
                    acc[ot][:, q0:q0 + qw],
                    acc[ot][:, q0:q0 + qw], ps[:, 0:qw])

        with nc.named_scope("qkv"):
            # dummy matmuls on the first-arrived DMA tile warm the PE's HAM
            # clock gate (~3.4us of activity) while the rest streams in;
            # then K(0), K(1), Q(0) keep the PE dense until the pipeline
            for w in range(8):
                wps = psA.tile([128, 512], f32, tag="psA", name=f"warm{w}")
                nc.tensor.matmul(wps[:, 0:512], xT[0][:, 0:128],
                                 xT[0][:, 0:512], start=True, stop=True)
            emit_k(0, [0, 1])
            emit_k(0, [2, 3])
            emit_q(0, [0, 1])

        # fill schedule: during pair p project Q(p+1) (tts 1, 5) and
        # K(p+1) (chunk-pairs at tts 2, 9 -- chunk pairs share the wqk
        # stationary so half the weight loads are skipped)
        fill = {p: {} for p in range(6)}
        for p in range(5):
            fill[p][1] = lambda ft=p + 1: emit_q(ft, [0])
            fill[p][5] = lambda ft=p + 1: emit_q(ft, [1])
            fill[p][2] = lambda ft=p + 1: emit_k(ft, [0, 1])
            fill[p][9] = lambda ft=p + 1: emit_k(ft, [2, 3])

        with nc.named_scope("attn"):
            # flat software pipeline: PV lags scores by 2 steps so the exp
            # (ACT engine) of steps s-1/s overlaps PV of step s-2
            pend = []       # (p, tt, ex, po_pair) awaiting PV
            prev_ev = None  # (p, pz_pair) awaiting normalize multiply
            prev_aq = None  # (p, aq_pair) awaiting transpose
            partials = []   # (j, k) proj contributions not yet emitted
            for p in range(6):
                po_pair = [psO.tile([128, 512], f32, tag=f"po{hh}",
                                    name=f"po{p}_{hh}") for hh in range(2)]
                if p >= 1:
                    partials.extend((p - 1, k) for k in range(12))
                for tt in range(13):
                    ex = emit_scores(p, tt)
                    # pair 0's first V slips one step so its wv DMA (late
                    # in the input stream) never blocks the PE FIFO
                    if tt % 4 == 0 and not (p == 0 and tt == 0):
                        emit_v(p, tt)
                    elif p == 0 and tt == 1:
                        emit_v(0, 0)
                    pend.append((p, tt, ex, po_pair))
                    # steady skew 2; shrink to 1 over the last two steps so
                    # the drain after the loop is short
                    max_pend = 1 if (p == 5 and tt >= 11) else 2
                    while len(pend) > max_pend:
                        pp, ptt, pex, ppo = pend.pop(0)
                        emit_pv(pp, ptt, pex, ppo)
                        if ptt == 12:
                            prev_ev = (pp, evict_po(pp, ppo))
                    if tt in fill[p]:
                        fill[p][tt]()
                    if tt == 2 and prev_ev is not None:
                        prev_aq = (prev_ev[0], emit_norm_mul(*prev_ev))
                        prev_ev = None
                    if tt == 3 and prev_aq is not None:
                        emit_norm_transpose(*prev_aq)
                        prev_aq = None
                    for _ in range(PARTIAL_POPS.get(tt, 0)):
                        if not partials:
                            break
                        jj, kk = partials[0]
                        # never emit a partial whose attn tile is written
                        # later in this same pair (PE FIFO would deadlock)
                        if tt in PARTIAL_SAFE_EARLY and jj == p - 1:
                            break
                        partials.pop(0)
                        emit_partial(jj, kk)
            # drain the pipeline: last pair's PV, normalize, proj, DMA out
            while pend:
                pp, ptt, pex, ppo = pend.pop(0)
                emit_pv(pp, ptt, pex, ppo)
                if ptt == 12:
                    prev_ev = (pp, evict_po(pp, ppo))
            aq5 = emit_norm_mul(*prev_ev)
            # the normalize chain leaves the PE idle just long enough for
            # the HAM clock gate to re-throttle; a few dummy matmuls into
            # the now-free po banks keep it at full clock for the final
            # projection matmuls
            for w in range(5):
                wps = psO.tile([128, 512], f32, tag="po0", name=f"warm2_{w}")
                nc.tensor.matmul(wps[:, 0:512], kT[0][0:128, 0:128],
                                 kT[0][:, 0:512], start=True, stop=True)
            emit_norm_transpose(prev_ev[0], aq5)
            for jj, kk in partials:      # leftover carried partials first
                emit_partial(jj, kk)
            # ot-major so each out tile's DMA streams while later tiles'
            # matmuls and adds are still running
            for ot in range(6):
                emit_partial(5, ot * 2)
                emit_partial(5, ot * 2 + 1)
                nc.sync.dma_start(out=out_d[ot * 128:(ot + 1) * 128, :],
                                  in_=acc[ot])

        phBn_cm.__exit__(None, None, None)
        phB_cm.__exit__(None, None, None)
        psA_cm.__exit__(None, None, None)
        psO_cm.__exit__(None, None, None)
        psS_cm.__exit__(None, None, None)
        phA_cm.__exit__(None, None, None)
        persist_cm.__exit__(None, None, None)

    nc.compile()
    return nc


def _get_program():
    if "nc" not in _cache:
        _cache["nc"] = _build_program()
    return _cache["nc"]


def _make_in_maps(x, qkv_w, q_bias, v_bias, proj_w, proj_b):
    wqk = np.ascontiguousarray(qkv_w[:, :2 * C])      # [C, 2C] (q cols, k cols)
    wv = np.ascontiguousarray(qkv_w[:, 2 * C:])       # [C, C]
    qb = np.zeros((128, 6), np.float32)
    qb[:, :] = q_bias.reshape(6, 128).T
    pb_eff = proj_b + v_bias @ proj_w                  # fold v_bias into proj
    pb = np.zeros((128, 6), np.float32)
    pb[:, :] = pb_eff.reshape(6, 128).T
    ident = np.eye(128, dtype=ml_dtypes.bfloat16)

    in_maps = []
    for c in range(N_CORES):
        b, half = c // 2, c % 2
        xr = np.ascontiguousarray(np.roll(x[b].T, -half * NQ, axis=1))
        in_maps.append({
            "xT": xr.astype(ml_dtypes.bfloat16),
            "wqk": wqk.astype(ml_dtypes.bfloat16),
            "wv": wv.astype(ml_dtypes.bfloat16),
            "wproj": proj_w.astype(ml_dtypes.bfloat16), "qb": qb, "pb": pb,
            "ident": ident,
        })
    return in_maps


def kernel(x, qkv_w, q_bias, v_bias, proj_w, proj_b):
    from concourse.bass_utils import run_bass_kernel_spmd

    x = np.asarray(x, dtype=np.float32)
    qkv_w = np.asarray(qkv_w, dtype=np.float32)
    q_bias = np.asarray(q_bias, dtype=np.float32)
    v_bias = np.asarray(v_bias, dtype=np.float32)
    proj_w = np.asarray(proj_w, dtype=np.float32)
    proj_b = np.asarray(proj_b, dtype=np.float32)

    nc = _get_program()
    in_maps = _make_in_maps(x, qkv_w, q_bias, v_bias, proj_w, proj_b)
    _cache["in_maps"] = in_maps

    res = run_bass_kernel_spmd(nc, in_maps, list(range(N_CORES)))
    out = np.empty((B, N, C), np.float32)
    for c in range(N_CORES):
        b, half = c // 2, c % 2
        out[b, half * NQ:(half + 1) * NQ, :] = res.results[c]["outT"].T
    return out


# revision 42
# speedup vs baseline: 1.0108x; 1.0108x over previous
"""Multi-head attention (B=4, N=1568, C=768, H=12) on 8 TRN2 NeuronCores.

Sharding: query-parallel. Core c handles batch b = c // 2 and query half
half = c % 2 (784 query tokens). Each core computes K/V projections for the
full 1568 tokens of its batch (duplicated across the pair), Q projection
for its 784 tokens, full attention for all 12 heads over its queries, and
the output projection. No cross-core communication.

Host-side tricks:
  - tokens are rotated per core so its own query half sits at columns 0:784
    of xT; the key order is then a (core-dependent) permutation, which
    softmax attention is invariant to.
  - v_bias is folded into the projection bias.
  - the softmax 1/sqrt(D) scale is folded into the exp activation's scale.

Device schedule (per core), heads in PAIRS (2ft, 2ft+1):
  - inputs arrive via a few LARGE consolidated DMAs (dma submission on the
    sync engine costs ~0.6us each); SBUF input tiles are split exactly at
    DMA boundaries so compute depends only on the pieces it reads
  - the head phase computes K(0), K(1), Q(0) back-to-back so the PE has no
    idle window (no HAM re-throttle) while the remaining weights stream in
  - flat software pipeline over (pair, key-tile) steps: scores(s) are
    emitted 2 steps ahead of PV(s), so the ACT-engine exp overlaps PV/V/
    projection matmuls and the PE never queue-blocks on the scalar engine
  - PV ("form B"): ex q-tiles are the matmul STATIONARY (128-wide tiles so
    the compiler's fast-weight-load halves the LDWEIGHTS cost; head A and
    head B tiles interleave so the weight-load pipeline never drains) and
    [V | ones] (65 cols) streams; col 64 gives the softmax denominator
    per-partition, normalized by a cheap reciprocal + tensor_scalar mult
  - normalize is split: the DVE multiply chain runs one step before the PE
    transposes so the PE FIFO never waits on the vector engine
  - output projection is INCREMENTAL: pair j's 12 rank-128 contributions
    are spread over later steps (a deque carries the overflow into the
    next pair, never scheduled before that pair's attn tile is written to
    keep the PE FIFO deadlock-free) and accumulated into SBUF f32
    accumulators on the vector engine (bias folded in at j==0), so only
    14 matmuls + two output DMAs remain after the last pair
  - per-step PSUM-scratch (psA, 2 banks) users are scheduled at most ~2
    allocations per step: V at tt 0/4/8/12, single K chunks at 2/6/9/11,
    Q chunks at 1/5, transposes at 3, partials fill the gaps
"""

import numpy as np
import ml_dtypes

B, N, C = 4, 1568, 768
H = 12
D = 64
NQ = N // 2          # 784 queries per core
SCALE = D ** -0.5
N_CORES = 8
KT = [128] * 12 + [32]          # key tiles (sum = 1568)
TCH = [(0, 392), (392, 392), (784, 392), (1176, 392)]  # token chunks (K/Q proj)
# query tiles for form-B PV: (ex column offset, width, output q offset).
# ex is laid out head-contiguous (A: cols 0:784, B: 784:1568); 6x128 + 16
# (128-wide stationaries trigger the compiler's fast weight load)
QT_A = [(128 * i, 128, 128 * i) for i in range(6)] + [(768, 16, 768)]
QT_B = [(784 + c, w, o) for (c, w, o) in QT_A]
# proj query chunks
PCH = [(0, 512), (512, 272)]
# how many pending proj partials to pop at each step tt. Slot 1 may only
# serve partials carried over from the PREVIOUS pair (attn of the current
# pair's j=p-1 is written at tt==3); capacity/pair = 12 with 1 carried.
PARTIAL_POPS = {1: 1, 4: 1, 5: 1, 6: 2, 7: 2, 8: 1, 10: 2, 12: 2}
PARTIAL_SAFE_EARLY = (1,)     # slots that may only serve carried partials

_cache = {}


def _build_program():
    import concourse.mybir as mybir
    from concourse import bacc
    from concourse.tile import TileContext

    f32 = mybir.dt.float32
    bf16 = mybir.dt.bfloat16
    Exp = mybir.ActivationFunctionType.Exp

    nc = bacc.Bacc("TRN2", target_bir_lowering=False, debug=False,
                   num_devices=N_CORES)

    xT_d = nc.dram_tensor("xT", [C, N], bf16, kind="ExternalInput")
    wqk_d = nc.dram_tensor("wqk", [C, 2 * C], bf16, kind="ExternalInput")
    wv_d = nc.dram_tensor("wv", [C, C], bf16, kind="ExternalInput")
    wp_d = nc.dram_tensor("wproj", [C, C], bf16, kind="ExternalInput")
    qb_d = nc.dram_tensor("qb", [128, 6], f32, kind="ExternalInput")
    pb_d = nc.dram_tensor("pb", [128, 6], f32, kind="ExternalInput")
    id_d = nc.dram_tensor("ident", [128, 128], bf16, kind="ExternalInput")
    out_d = nc.dram_tensor("outT", [C, NQ], f32, kind="ExternalOutput")

    with TileContext(nc) as tc:
        persist_cm = tc.tile_pool(name="persist", bufs=1)
        persist = persist_cm.__enter__()
        kT = [persist.tile([128, N], bf16, tag=f"kT{j}", name=f"kT{j}")
              for j in range(6)]
        qT = [persist.tile([128, NQ], bf16, tag=f"qT{j}", name=f"qT{j}")
              for j in range(6)]
        # V for all 13 key tiles: [tt][pair][head-of-pair][65] along free dim
        vbig = persist.tile([128, 13 * 780], bf16, tag="vbig", name="vbig")
        attn = [persist.tile([128, NQ], bf16, tag=f"at{j}", name=f"at{j}")
                for j in range(6)]
        # incremental output-projection accumulators (f32); two tiles of
        # three out-blocks each so the two output DMAs depend on halves
        acc_t = [persist.tile([128, 3 * NQ], f32, tag=f"ac{h}", name=f"ac{h}")
                 for h in range(2)]
        acc_v = [t.rearrange("p (o n) -> p o n", o=3) for t in acc_t]
        acc = [acc_v[ot // 3][:, ot % 3, :] for ot in range(6)]
        qb_sb = persist.tile([128, 6], f32, tag="qb")
        pb_sb = persist.tile([128, 6], f32, tag="pb")
        id_sb = persist.tile([128, 128], bf16, tag="ident")

        # input tiles split exactly at DMA granularity
        phA_cm = tc.tile_pool(name="phA", bufs=1)
        phA = phA_cm.__enter__()
        xT_t = [phA.tile([128, N], bf16, tag="xTa", name="xTsa"),
                phA.tile([128, N], bf16, tag="xTb", name="xTsb"),
                phA.tile([128, 2 * N], bf16, tag="xTc", name="xTsc"),
                phA.tile([128, 2 * N], bf16, tag="xTd", name="xTsd")]
        xT_v = [t.rearrange("p (j n) -> p j n", j=max(1, t.shape[1] // N))
                for t in xT_t]
        _xmap = [(0, 0), (1, 0), (2, 0), (2, 1), (3, 0), (3, 1)]
        xT = [xT_v[a][:, b, :] for (a, b) in _xmap]
        wqkK_t = [phA.tile([128, 3 * C], bf16, tag=f"wK{h}", name=f"wKs{h}")
                  for h in range(2)]
        wqkK_v = [t.rearrange("p (j n) -> p j n", j=3) for t in wqkK_t]
        wK = [wqkK_v[j // 3][:, j % 3, :] for j in range(6)]
        wqkQ_t = [phA.tile([128, 3 * C], bf16, tag=f"wQ{h}", name=f"wQs{h}")
                  for h in range(2)]
        wqkQ_v = [t.rearrange("p (j n) -> p j n", j=3) for t in wqkQ_t]
        wQ = [wqkQ_v[j // 3][:, j % 3, :] for j in range(6)]
        wv_big = phA.tile([128, 6 * C], bf16, tag="wv", name="wvs")
        wvv = wv_big.rearrange("p (j n) -> p j n", j=6)
        wv = [wvv[:, j, :] for j in range(6)]
        wp_big = phA.tile([128, 6 * C], bf16, tag="wp", name="wps")
        wpv = wp_big.rearrange("p (j n) -> p j n", j=6)
        wp_sb = [wpv[:, j, :] for j in range(6)]

        def dma_rows(dst_view, dram, r0, r1, c0=None, c1=None):
            src = dram[r0:r1, :] if c0 is None else dram[r0:r1, c0:c1]
            nc.sync.dma_start(
                out=dst_view, in_=src.rearrange("(j p) n -> p j n", p=128))

        # DMA order = consumption order; few large transfers (bandwidth
        # bound) instead of many small ones (submission bound). The first
        # xT block is small so the HAM warmup starts as early as possible.
        dma_rows(xT_v[0], xT_d, 0, 128)
        dma_rows(wqkK_v[0], wqk_d, 0, 384, C, 2 * C)
        dma_rows(xT_v[1], xT_d, 128, 256)
        dma_rows(xT_v[2], xT_d, 256, 512)
        dma_rows(wqkK_v[1], wqk_d, 384, 768, C, 2 * C)
        dma_rows(xT_v[3], xT_d, 512, 768)
        dma_rows(wqkQ_v[0], wqk_d, 0, 384, 0, C)
        dma_rows(wqkQ_v[1], wqk_d, 384, 768, 0, C)
        nc.sync.dma_start(out=qb_sb, in_=qb_d[:])
        nc.sync.dma_start(out=id_sb, in_=id_d[:])
        nc.sync.dma_start(out=pb_sb, in_=pb_d[:])
        dma_rows(wvv[:, :, :], wv_d, 0, 768)   # lands before the first V use
        dma_rows(wpv[:, :, :], wp_d, 0, 768)   # needed ~1 pair in
        # ones columns: every 65th col of vbig starting at 64
        nc.vector.memset(
            vbig.rearrange("p (t e) -> p t e", e=65)[:, :, 64:65], 1.0)

        # PSUM pools: sJ0 2 + sJ1 2 + po 2 + psA 2 = 8 banks
        psS_cm = tc.tile_pool(name="psS", bufs=1, space="PSUM")
        psS = psS_cm.__enter__()
        psO_cm = tc.tile_pool(name="psO", bufs=1, space="PSUM")
        psO = psO_cm.__enter__()
        psA_cm = tc.tile_pool(name="psA", bufs=2, space="PSUM")
        psA = psA_cm.__enter__()
        phB_cm = tc.tile_pool(name="phB", bufs=5)
        phB = phB_cm.__enter__()
        phBn_cm = tc.tile_pool(name="phBn", bufs=3)
        phBn = phBn_cm.__enter__()

        def emit_k(ft, chunks):
            # chunk-group inner loop: consecutive matmuls share the wqk
            # stationary so later chunks skip their weight load
            pss = [psA.tile([128, 512], f32, tag="psA", name=f"k{ft}_{ci}")
                   for ci in chunks]
            for j in range(6):
                for ps, ci in zip(pss, chunks):
                    (t0, tw) = TCH[ci]
                    nc.tensor.matmul(
                        ps[:, 0:tw],
                        wK[j][:, ft * 128:ft * 128 + 128],
                        xT[j][:, t0:t0 + tw],
                        start=(j == 0), stop=(j == 5),
                        skip_group_check=True,
                    )
            for ps, ci in zip(pss, chunks):
                (t0, tw) = TCH[ci]
                nc.vector.tensor_copy(kT[ft][:, t0:t0 + tw], ps[:, 0:tw])

        def emit_q(ft, chunks):
            pss = [psA.tile([128, 512], f32, tag="psA", name=f"q{ft}_{ci}")
                   for ci in chunks]
            for j in range(6):
                for ps, ci in zip(pss, chunks):
                    (t0, tw) = TCH[ci]
                    nc.tensor.matmul(
                        ps[:, 0:tw],
                        wQ[j][:, ft * 128:ft * 128 + 128],
                        xT[j][:, t0:t0 + tw],
                        start=(j == 0), stop=(j == 5),
                        skip_group_check=True,
                    )
            for ps, ci in zip(pss, chunks):
                (t0, tw) = TCH[ci]
                nc.vector.tensor_scalar(
                    out=qT[ft][:, t0:t0 + tw], in0=ps[:, 0:tw],
                    scalar1=qb_sb[:, ft:ft + 1], scalar2=None,
                    op0=mybir.AluOpType.add,
                )

        def emit_v(p, tt0):
            # V for pair p, key tiles tt0..tt0+3 (4-tile batch), into one
            # psA tile then one strided eviction into vbig
            tts = [t for t in range(tt0, min(tt0 + 4, 13))]
            ps = psA.tile([128, 512], f32, tag="psA", name=f"v{p}_{tt0}")
            for i, tt in enumerate(tts):
                mt = KT[tt]
                for j in range(6):
                    nc.tensor.matmul(
                        ps[0:mt, i * 128:i * 128 + 128],
                        xT[j][:, tt * 128:tt * 128 + mt],
                        wv[j][:, p * 128:(p + 1) * 128],
                        start=(j == 0 and i == 0),
                        stop=(j == 5 and i == len(tts) - 1),
                        skip_group_check=True,
                    )
            src = ps.rearrange("q (i h e) -> q i h e", i=4, h=2)[
                0:128, 0:len(tts), :, :]
            dst = vbig.rearrange("q (t pp h e) -> q t pp h e", pp=6, h=2, e=65)[
                0:128, tt0:tt0 + len(tts), p, :, 0:64]
            nc.vector.tensor_copy(dst, src)

        def emit_scores(ft, tt):
            mt = KT[tt]
            sJ0 = psS.tile([128, 1024], f32, tag="sJ0", name=f"s0_{ft}_{tt}")
            sJ1 = psS.tile([128, 1024], f32, tag="sJ1", name=f"s1_{ft}_{tt}")
            ks = slice(tt * 128, tt * 128 + mt)
            nc.tensor.matmul(sJ0[0:mt, 0:512], kT[ft][0:64, ks],
                             qT[ft][0:64, 0:512], start=True, stop=True)
            nc.tensor.matmul(sJ1[0:mt, 0:272], kT[ft][0:64, ks],
                             qT[ft][0:64, 512:784], start=True, stop=True)
            nc.tensor.matmul(sJ0[0:mt, 512:1024], kT[ft][64:128, ks],
                             qT[ft][64:128, 0:512], start=True, stop=True)
            nc.tensor.matmul(sJ1[0:mt, 512:784], kT[ft][64:128, ks],
                             qT[ft][64:128, 512:784], start=True, stop=True)
            # ex layout: head A = cols 0:784, head B = cols 784:1568
            ex = phB.tile([128, 1568], bf16, tag="ex", name=f"ex{ft}_{tt}")
            ex2 = ex.rearrange("p (b q) -> p b q", b=2)
            nc.scalar.activation(out=ex2[0:mt, :, 0:512],
                                 in_=sJ0.rearrange("p (b q) -> p b q", b=2)[0:mt],
                                 func=Exp, scale=SCALE)
            nc.scalar.activation(
                out=ex2[0:mt, :, 512:784],
                in_=sJ1.rearrange("p (b q) -> p b q", b=2)[0:mt, :, 0:272],
                func=Exp, scale=SCALE)
            return ex

        def emit_pv(p, tt, ex, po_pair):
            mt = KT[tt]
            v5 = vbig.rearrange("q (t pp h e) -> q t pp h e", pp=6, h=2, e=65)
            vh = [v5[0:mt, tt, p, hh, :] for hh in range(2)]
            # interleave head A / head B q-tiles so the PE's background
            # weight-load pipeline never drains at a head boundary
            for i in range(7):
                for hh in range(2):
                    (c0, qw, _) = (QT_A if hh == 0 else QT_B)[i]
                    nc.tensor.matmul(
                        po_pair[hh][0:qw, i * 65:(i + 1) * 65],
                        ex[0:mt, c0:c0 + qw],
                        vh[hh],
                        start=(tt == 0 and i == 0),
                        stop=(tt == 12 and i == 6),
                        skip_group_check=True,
                    )

        def evict_po(p, po_pair):
            # the last pair's eviction goes on the (by then idle) scalar
            # engine, freeing the vector engine for the normalize chain; in
            # steady state the scalar engine is busy with exps (strict FIFO
            # queue -- a copy there would delay them), so DVE does it
            pz_pair = []
            for hh in range(2):
                pz = phBn.tile([128, 455], f32, tag=f"pz{hh}",
                               name=f"pz{p}_{hh}")
                if p == 5:
                    nc.scalar.copy(pz, po_pair[hh][:, 0:455])
                else:
                    nc.vector.tensor_copy(pz, po_pair[hh][:, 0:455])
                pz_pair.append(pz)
            return pz_pair

        def emit_norm_mul(p, pz_pair):
            # pz[q, 65i:65i+64] = PV, pz[q, 65i+64] = denominator (per q!)
            aq_pair = []
            for hh in range(2):
                pz = pz_pair[hh]
                qts = QT_A if hh == 0 else QT_B
                rec = phBn.tile([128, 7], f32, tag=f"rec{hh}", name=f"rc{p}_{hh}")
                nc.vector.reciprocal_approx_fast(
                    out=rec,
                    in_=pz.rearrange("q (i e) -> q i e", e=65)[:, 0:7, 64])
                aq = phBn.tile([128, 448], bf16, tag=f"aq{hh}",
                               name=f"aq{p}_{hh}")
                # one multiply for all 7 q-tiles: the reciprocal broadcasts
                # across each tile's 64 feature columns via a 0-stride AP
                # (garbage beyond the 16 valid partitions of the last tile
                # is never read downstream)
                nc.vector.tensor_tensor(
                    aq.rearrange("q (i e) -> q i e", e=64),
                    pz.rearrange("q (i e) -> q i e", e=65)[:, :, 0:64],
                    rec.rearrange("q (i o) -> q i o", o=1)
                    .broadcast_to([128, 7, 64]),
                    mybir.AluOpType.mult)
                aq_pair.append(aq)
            return aq_pair

        def emit_norm_transpose(p, aq_pair):
            # transpose [q, 64] tiles back to feature-major via PE
            for hh in range(2):
                ft, fo = p, hh * 64
                aq = aq_pair[hh]
                qts = QT_A if hh == 0 else QT_B
                for g0, gn, q0 in ((0, 4, 0), (4, 3, 512)):
                    pt = psA.tile([128, 512], bf16, tag="psA",
                                  name=f"tr{p}_{hh}_{g0}")
                    for i in range(g0, g0 + gn):
                        qw = qts[i][1]
                        qo = qts[i][2] - q0
                        nc.tensor.transpose(
                            pt[0:64, qo:qo + qw],
                            aq[0:qw, i * 64:i * 64 + 64],
                            id_sb[0:qw, 0:qw])
                    gw = qts[g0 + gn - 1][2] + qts[g0 + gn - 1][1] - q0
                    nc.vector.tensor_copy(
                        attn[ft][fo:fo + 64, q0:q0 + gw], pt[0:64, 0:gw])

        def emit_partial(j, k):
            # proj contribution of attn feature-tile j to out tile ot,
            # query chunk (q0, qw); accumulated on DVE into acc[ot]
            ot, ci = k // 2, k % 2
            (q0, qw) = PCH[ci]
            ps = psA.tile([128, 512], f32, tag="psA", name=f"pp{j}_{k}")
            nc.tensor.matmul(
                ps[:, 0:qw],
                wp_sb[j][:, ot * 128:(ot + 1) * 128],
                attn[j][:, q0:q0 + qw],
                start=True, stop=True,
                skip_group_check=True,
            )
            if j == 0:
                nc.vector.tensor_scalar(
                    out=acc[ot][:, q0:q0 + qw], in0=ps[:, 0:qw],
                    scalar1=pb_sb[:, ot:ot + 1], scalar2=None,
                    op0=mybir.AluOpType.add,
                )
            else:
                nc.vector.tensor_add(
                    acc[ot][:, q0:q0 + qw],
                    acc[ot][:, q0:q0 + qw], ps[:, 0:qw])

        with nc.named_scope("qkv"):
            # dummy matmuls on the first-arrived DMA tile warm the PE's HAM
            # clock gate (~3.4us of activity) while the rest streams in;
            # then K(0), K(1), Q(0) keep the PE dense until the pipeline
            for w in range(8):
                wps = psA.tile([128, 512], f32, tag="psA", name=f"warm{w}")
                nc.tensor.matmul(wps[:, 0:512], xT[0][:, 0:128],
                                 xT[0][:, 0:512], start=True, stop=True)
            emit_k(0, [0, 1])
            emit_k(0, [2, 3])
            emit_q(0, [0, 1])

        # fill schedule: during pair p project Q(p+1) (tts 1, 5) and
        # K(p+1) (chunk-pairs at tts 2, 9 -- chunk pairs share the wqk
        # stationary so half the weight loads are skipped)
        fill = {p: {} for p in range(6)}
        for p in range(5):
            fill[p][1] = lambda ft=p + 1: emit_q(ft, [0])
            fill[p][5] = lambda ft=p + 1: emit_q(ft, [1])
            fill[p][2] = lambda ft=p + 1: emit_k(ft, [0, 1])
            fill[p][9] = lambda ft=p + 1: emit_k(ft, [2, 3])

        with nc.named_scope("attn"):
            # flat software pipeline: PV lags scores by 2 steps so the exp
            # (ACT engine) of steps s-1/s overlaps PV of step s-2
            pend = []       # (p, tt, ex, po_pair) awaiting PV
            prev_ev = None  # (p, pz_pair) awaiting normalize multiply
            prev_aq = None  # (p, aq_pair) awaiting transpose
            partials = []   # (j, k) proj contributions not yet emitted
            for p in range(6):
                po_pair = [psO.tile([128, 512], f32, tag=f"po{hh}",
                                    name=f"po{p}_{hh}") for hh in range(2)]
                if p >= 1:
                    partials.extend((p - 1, k) for k in range(12))
                for tt in range(13):
                    ex = emit_scores(p, tt)
                    # pair 0's first V slips one step so its wv DMA (late
                    # in the input stream) never blocks the PE FIFO
                    if tt % 4 == 0 and not (p == 0 and tt == 0):
                        emit_v(p, tt)
                    elif p == 0 and tt == 1:
                        emit_v(0, 0)
                    pend.append((p, tt, ex, po_pair))
                    # steady skew 2; shrink to 1 over the last two steps so
                    # the drain after the loop is short
                    max_pend = 1 if (p == 5 and tt >= 11) else 2
                    while len(pend) > max_pend:
                        pp, ptt, pex, ppo = pend.pop(0)
                        emit_pv(pp, ptt, pex, ppo)
                        if ptt == 12:
                            prev_ev = (pp, evict_po(pp, ppo))
                    if tt in fill[p]:
                        fill[p][tt]()
                    if tt == 2 and prev_ev is not None:
                        prev_aq = (prev_ev[0], emit_norm_mul(*prev_ev))
                        prev_ev = None
                    if tt == 3 and prev_aq is not None:
                        emit_norm_transpose(*prev_aq)
                        prev_aq = None
                    for _ in range(PARTIAL_POPS.get(tt, 0)):
                        if not partials:
                            break
                        jj, kk = partials[0]
                        # never emit a partial whose attn tile is written
                        # later in this same pair (PE FIFO would deadlock)
                        if tt in PARTIAL_SAFE_EARLY and jj == p - 1:
                            break
                        partials.pop(0)
                        emit_partial(jj, kk)
            # drain the pipeline: last pair's PV, normalize, proj, DMA out
            while pend:
                pp, ptt, pex, ppo = pend.pop(0)
                emit_pv(pp, ptt, pex, ppo)
                if ptt == 12:
                    prev_ev = (pp, evict_po(pp, ppo))
            emit_norm_transpose(prev_ev[0], emit_norm_mul(*prev_ev))
            for jj, kk in partials:      # leftover carried partials first
                emit_partial(jj, kk)
            # ot-major so each out tile's DMA streams while later tiles'
            # matmuls and adds are still running
            for ot in range(6):
                emit_partial(5, ot * 2)
                emit_partial(5, ot * 2 + 1)
                nc.sync.dma_start(out=out_d[ot * 128:(ot + 1) * 128, :],
                                  in_=acc[ot])

        phBn_cm.__exit__(None, None, None)
        phB_cm.__exit__(None, None, None)
        psA_cm.__exit__(None, None, None)
        psO_cm.__exit__(None, None, None)
        psS_cm.__exit__(None, None, None)
        phA_cm.__exit__(None, None, None)
        persist_cm.__exit__(None, None, None)

    nc.compile()
    return nc


def _get_program():
    if "nc" not in _cache:
        _cache["nc"] = _build_program()
    return _cache["nc"]


def _make_in_maps(x, qkv_w, q_bias, v_bias, proj_w, proj_b):
    wqk = np.ascontiguousarray(qkv_w[:, :2 * C])      # [C, 2C] (q cols, k cols)
    wv = np.ascontiguousarray(qkv_w[:, 2 * C:])       # [C, C]
    qb = np.zeros((128, 6), np.float32)
    qb[:, :] = q_bias.reshape(6, 128).T
    pb_eff = proj_b + v_bias @ proj_w                  # fold v_bias into proj
    pb = np.zeros((128, 6), np.float32)
    pb[:, :] = pb_eff.reshape(6, 128).T
    ident = np.eye(128, dtype=ml_dtypes.bfloat16)

    in_maps = []
    for c in range(N_CORES):
        b, half = c // 2, c % 2
        xr = np.ascontiguousarray(np.roll(x[b].T, -half * NQ, axis=1))
        in_maps.append({
            "xT": xr.astype(ml_dtypes.bfloat16),
            "wqk": wqk.astype(ml_dtypes.bfloat16),
            "wv": wv.astype(ml_dtypes.bfloat16),
            "wproj": proj_w.astype(ml_dtypes.bfloat16), "qb": qb, "pb": pb,
            "ident": ident,
        })
    return in_maps


def kernel(x, qkv_w, q_bias, v_bias, proj_w, proj_b):
    from concourse.bass_utils import run_bass_kernel_spmd

    x = np.asarray(x, dtype=np.float32)
    qkv_w = np.asarray(qkv_w, dtype=np.float32)
    q_bias = np.asarray(q_bias, dtype=np.float32)
    v_bias = np.asarray(v_bias, dtype=np.float32)
    proj_w = np.asarray(proj_w, dtype=np.float32)
    proj_b = np.asarray(proj_b, dtype=np.float32)

    nc = _get_program()
    in_maps = _make_in_maps(x, qkv_w, q_bias, v_bias, proj_w, proj_b)
    _cache["in_maps"] = in_maps

    res = run_bass_kernel_spmd(nc, in_maps, list(range(N_CORES)))
    out = np.empty((B, N, C), np.float32)
    for c in range(N_CORES):
        b, half = c // 2, c % 2
        out[b, half * NQ:(half + 1) * NQ, :] = res.results[c]["outT"].T
    return out
